# revision 1
# baseline (speedup 1.0000x reference)
"""Distributed masked-attention kernel for 8 TRN2 NeuronCores.

Problem: single-head attention, N=4 batches, S=4096, E=512 (f32), with an
elementwise int32 0/1 mask on the [S, S] score matrix.

Sharding: 8 shards = (batch b, query-half h); each core handles 2048 queries
of one batch against all 4096 keys of that batch. Fully data-parallel, no
collectives.

Everything on device runs in the "transposed" domain so the TensorEngine
never needs an on-chip transpose, with the projection weights folded
host-side (associativity only -- O(E^3) work):
  - scoresT[j, i] = kT.T @ q~T, where q~ = q (Wq'.T Wk) folds BOTH
    score-side projections into one [512,512] matrix, so raw (transposed)
    keys feed the score matmul directly.
  - attnT[j, i]   = exp(scoresT) * maskT  (multiplicative 0/1 bf16 mask,
    identical to the -inf additive bias since exp(s)*m == exp(s + log m)
    for m in {0,1}; |scores| <~ 6 so exp never overflows).
  - out1T[f, i]   = V.T-chunks @ attnT: raw V rows as the stationary
    operand, attn as the moving operand -- the whole query quarter
    accumulates across all 32 key tiles in 4 PSUM banks.
  - out[i, e]     = (out1T / denom).T-chunks @ W2, W2 = (Wo Wv).T applied
    AFTER the attention contraction (2048 rows instead of 4096 -- half the
    projection work of projecting V).
  - denom[i] = sum_j attnT[j, i]: the otherwise-idle GPSIMD engine
    accumulates attn tiles into a per-partition partial [128, i]; a
    1-row f32 ones-matmul finishes the cross-partition sum and tiny PE
    transposes flip [1,128] chunks onto partitions, so 1/denom applies
    as a per-partition scalar fused into the drain (mult+bias-add in one
    DVE op). The drain order keeps the PE stream dense so the HAM clock
    gate never re-throttles mid-epilogue.

All device-side tensors arrive as bf16 (host-side cast -- numerically
identical to the on-chip cast it replaces, and it halves HBM traffic).
PE compute is bf16 (fp8 was tested and rejected: attention-weight
quantization error propagates to the output at full per-element
magnitude). Output is written bf16 and upcast on host. DRAM inputs are
laid out host-side so each DMA descriptor moves KB-contiguous runs per
partition.
"""

import sys

import numpy as np
import ml_dtypes

if "/opt/trn_rl_repo" not in sys.path:
    sys.path.insert(0, "/opt/trn_rl_repo")

import concourse.bass as bass
import concourse.tile as tile
from concourse import mybir
from concourse.bass_utils import run_bass_kernel_spmd

F32 = mybir.dt.float32
BF16 = mybir.dt.bfloat16
BF = ml_dtypes.bfloat16

N, S, E = 4, 4096, 512
P = 128
QH = S // 2          # queries per core
ED = E // P          # 4 chunks of the embedding dim
JT = S // P          # 32 key tiles
NQ = 4               # i-quarters per core
IQW = QH // NQ       # 512 queries per quarter
IC = IQW // P        # 4 i-chunks per quarter
KSPAN = 512          # j-span for streaming k/q through the prologue
NCORES = 8


def build_bass():
    nc = bass.Bass()

    # layouts are pre-tiled on host: [span/group, 128, chunk, width]
    qT = nc.declare_dram_parameter("qT", [QH // KSPAN, P, ED, KSPAN], BF16, isOutput=False)
    kT = nc.declare_dram_parameter("kT", [S // KSPAN, P, ED, KSPAN], BF16, isOutput=False)
    vN = nc.declare_dram_parameter("vN", [P, JT, KSPAN], BF16, isOutput=False)
    maskT = nc.declare_dram_parameter("maskT", [NQ, 8, P, 4, IQW], BF16, isOutput=False)
    wqk = nc.declare_dram_parameter("wqk", [P, ED, E], BF16, isOutput=False)
    w2T = nc.declare_dram_parameter("w2T", [P, ED, E], BF16, isOutput=False)
    bo = nc.declare_dram_parameter("bo", [P, E], F32, isOutput=False)
    out = nc.declare_dram_parameter("out", [QH, E], BF16, isOutput=True)

    with tile.TileContext(nc) as tc:
        with (
            tc.tile_pool(name="persist", bufs=1) as persist,
            tc.tile_pool(name="xload", bufs=3) as xload,
            tc.tile_pool(name="maskp", bufs=4) as maskp,
            tc.tile_pool(name="expp", bufs=3) as expp,
            tc.tile_pool(name="attnp", bufs=6) as attnp,
            tc.tile_pool(name="denp", bufs=2) as denp,
            tc.tile_pool(name="o1p", bufs=2) as o1p,
            tc.tile_pool(name="outp", bufs=3) as outp,
            tc.tile_pool(name="ps_s", bufs=2, space="PSUM") as ps_s,
            tc.tile_pool(name="ps_b1", bufs=1, space="PSUM") as ps_b1,
            tc.tile_pool(name="ps_pp", bufs=2, space="PSUM") as ps_pp,
        ):
            # persistent tensors (bf16)
            wqk_b = persist.tile([P, ED, E], BF16)
            w2_b = persist.tile([P, ED, E], BF16)
            bo_sb = persist.tile([P, E], F32)
            kb_sb = persist.tile([P, ED, S], BF16)      # raw kT [d, j]
            qp_sb = persist.tile([P, ED, QH], BF16)     # q~T  [d, i]
            v_sb = persist.tile([P, JT, KSPAN], BF16)   # raw V [j, f]

            def emit_qproj(qs, qb=None):
                if qb is None:
                    qb = xload.tile([P, ED, KSPAN], BF16, tag="xb")
                    nc.sync.dma_start(out=qb, in_=qT[qs])
                for ec in range(ED):
                    ps = ps_pp.tile([P, KSPAN], F32, tag="pp")
                    for dc in range(ED):
                        nc.tensor.matmul(
                            out=ps,
                            lhsT=wqk_b[:, dc, ec * P:(ec + 1) * P],
                            rhs=qb[:, dc, :],
                            start=(dc == 0),
                            stop=(dc == ED - 1),
                        )
                    nc.scalar.copy(
                        out=qp_sb[:, ec, qs * KSPAN:(qs + 1) * KSPAN],
                        in_=ps,
                    )

            # issue the first weight + query DMAs immediately, then bridge
            # the transfer window with dummy wide matmuls on a zeroed tile:
            # they ramp the HAM clock gate (~3.4us of sustained activity)
            # and keep the PE dense so qproj and the first strips run warm
            # instead of cold with re-throttle episodes.
            qb0 = xload.tile([P, ED, KSPAN], BF16, tag="xb")
            for dc in range(ED):
                nc.sync.dma_start(out=wqk_b[:, dc, :], in_=wqk[:, dc, :])
                nc.sync.dma_start(out=qb0[:, dc, :], in_=qT[0, :, dc, :])
            # first K/V spans and mask group also start now (everything
            # else prefetches one group ahead inside the loop)
            for js0 in (0, 1):
                nc.sync.dma_start(
                    out=kb_sb[:, :, js0 * KSPAN:(js0 + 1) * KSPAN],
                    in_=kT[js0],
                )
                nc.sync.dma_start(
                    out=v_sb[:, js0 * 4:(js0 + 1) * 4, :],
                    in_=vN[:, js0 * 4:(js0 + 1) * 4, :],
                )

            def mask_dma(q_, g_):
                mt = maskp.tile([P, 4, IQW], BF16, tag="mask")
                nc.sync.dma_start(out=mt, in_=maskT[q_, g_])
                return mt

            pending_mask = mask_dma(0, 0)

            dm = nc.const_aps.tensor(1.0, (P, KSPAN), BF16)
            for i in range(20):
                psd = ps_pp.tile([P, KSPAN], F32, tag="pp", name=f"warm_{i}")
                nc.tensor.matmul(out=psd, lhsT=dm[:, 0:P], rhs=dm[:, :],
                                 start=True, stop=True)
            emit_qproj(0, qb=qb0)

            def emit_strip(q, jt, mtiles, at_tiles, den):
                ps = ps_s.tile([P, IQW], F32, tag="ps_s")
                for dc in range(ED):
                    nc.tensor.matmul(
                        out=ps,
                        lhsT=kb_sb[:, dc, jt * P:(jt + 1) * P],
                        rhs=qp_sb[:, dc, q * IQW:(q + 1) * IQW],
                        start=(dc == 0),
                        stop=(dc == ED - 1),
                    )
                ex = expp.tile([P, IQW], BF16, tag="ex")
                nc.scalar.activation(
                    out=ex, in_=ps, func=mybir.ActivationFunctionType.Exp
                )
                at = attnp.tile([P, IQW], BF16, tag="at")
                nc.vector.tensor_mul(
                    out=at, in0=ex, in1=mtiles[jt // 4][:, jt % 4, :]
                )
                at_tiles.append(at)
                # denominator partials ride the otherwise-idle GPSIMD
                if jt == 0:
                    nc.gpsimd.tensor_copy(out=den, in_=at)
                else:
                    nc.gpsimd.tensor_add(out=den, in0=den, in1=at)

            def bp_mms(jt, at, po1):
                for fc in range(ED):
                    nc.tensor.matmul(
                        out=po1[fc],
                        lhsT=v_sb[:, jt, fc * P:(fc + 1) * P],
                        rhs=at,
                        start=(jt == 0),
                        stop=(jt == JT - 1),
                    )

            ones_f32 = nc.const_aps.tensor(1.0, (P, 1), F32)
            id11 = nc.const_aps.tensor(1.0, (1, 1), F32)

            def emit_post_head(q, po1, den):
                # issued at the next quarter's start: ACT/DVE are idle at
                # the boundary, so the PSUM drain copies finish before the
                # PE needs the po1 banks back for the new quarter
                o1sb = []
                for fc in range(ED):
                    t = o1p.tile([P, IQW], BF16, tag=f"o1sb{fc}")
                    # split PSUM drain copies across ACT and DVE
                    if fc % 2 == 0:
                        nc.scalar.copy(out=t, in_=po1[fc])
                    else:
                        nc.vector.tensor_copy(out=t, in_=po1[fc])
                    o1sb.append(t)
                return q, o1sb, den

            def emit_post(q, o1sb, den):
                # the ones-matmul slots in AFTER the first postproj group:
                # for quarters 0-2 (emitted at jt==3) the denominator has
                # long finished either way, and for the final quarter this
                # keeps the PE on postproj matmuls instead of stalling on
                # the GPSIMD denominator chain right after the last B'.
                dps = den_row = None
                # postproj matmuls only need o1sb (the normalization folds
                # into the drain STT); the denominator transposes and
                # reciprocals interleave between postproj groups to keep
                # the PE dense so HAM never re-throttles.
                r_all = denp.tile([P, IC], F32, tag="rall")

                def den_T(ic):
                    # transpose lands in spare columns of dps (row 0 was
                    # already copied out to den_row, so overwriting is safe
                    # and the WAR dependency orders it after the copy)
                    col = 508 + ic
                    nc.tensor.transpose(
                        out=dps[:, col:col + 1],
                        in_=den_row[0:1, ic * P:(ic + 1) * P],
                        identity=id11,
                    )
                    nc.vector.reciprocal(
                        out=r_all[:, ic:ic + 1], in_=dps[:, col:col + 1]
                    )

                for ic in range(IC):
                    out_sb = outp.tile([P, E], BF16, tag="out")
                    ps2 = ps_pp.tile([P, E], F32, tag="pp")
                    for fc in range(ED):
                        nc.tensor.matmul(
                            out=ps2,
                            lhsT=o1sb[fc][:, ic * P:(ic + 1) * P],
                            rhs=w2_b[:, fc, :],
                            start=(fc == 0),
                            stop=(fc == ED - 1),
                        )
                    if ic == 0:
                        dps = ps_s.tile([P, IQW], F32, tag="ps_s",
                                        name=f"dps_{q}")
                        nc.tensor.matmul(out=dps[0:1, :], lhsT=ones_f32,
                                         rhs=den, start=True, stop=True)
                        den_row = denp.tile([1, IQW], F32, tag="drow")
                        nc.vector.tensor_copy(out=den_row, in_=dps[0:1, :])
                    # weave denominator transposes between postproj groups
                    # so the PE never idles long enough to re-throttle
                    if ic < 2:
                        den_T(2 * ic)
                        den_T(2 * ic + 1)
                    nc.vector.scalar_tensor_tensor(
                        out=out_sb, in0=ps2, scalar=r_all[:, ic:ic + 1],
                        in1=bo_sb,
                        op0=mybir.AluOpType.mult,
                        op1=mybir.AluOpType.add,
                    )
                    nc.sync.dma_start(
                        out=out[(q * IC + ic) * P:(q * IC + ic + 1) * P, :],
                        in_=out_sb,
                    )

            # ------------- fused main pipeline over query quarters ---------
            pending = None
            for q in range(NQ):
                mtiles = []
                at_tiles = []
                if pending is not None:
                    pending = emit_post_head(*pending)
                po1 = [
                    ps_b1.tile([P, IQW], F32, tag=f"o1_{fc}",
                               name=f"o1_{q}_{fc}")
                    for fc in range(ED)
                ]
                den = denp.tile([P, IQW], F32, tag="den")
                for jt in range(JT):
                    js = jt // 4
                    if jt % 4 == 0:
                        if q == 0:
                            if js < 6:
                                # K/V spans prefetch two groups ahead
                                ns_ = js + 2
                                nc.sync.dma_start(
                                    out=kb_sb[:, :,
                                              ns_ * KSPAN:(ns_ + 1) * KSPAN],
                                    in_=kT[ns_],
                                )
                                nc.sync.dma_start(
                                    out=v_sb[:, ns_ * 4:(ns_ + 1) * 4, :],
                                    in_=vN[:, ns_ * 4:(ns_ + 1) * 4, :],
                                )
                            if js == 1:
                                # W2/bias are not needed until this
                                # quarter's postproj -- keep them out of
                                # the critical first-strip DMA window
                                nc.sync.dma_start(out=w2_b, in_=w2T[:, :, :])
                                nc.sync.dma_start(out=bo_sb, in_=bo[:, :])
                            if js in (2, 4, 6):
                                emit_qproj(js // 2)
                        if q == 0 and js == 0:
                            mtiles.append(pending_mask)
                        else:
                            mtiles.append(mask_dma(q, js))
                    if pending is not None and jt == 3:
                        emit_post(*pending)
                        pending = None
                    # 2-deep software pipeline: scores for jt issue before
                    # jt-2's B matmuls so exp/mask-mul latency is hidden
                    emit_strip(q, jt, mtiles, at_tiles, den)
                    if jt >= 2:
                        bp_mms(jt - 2, at_tiles[jt - 2], po1)
                bp_mms(JT - 2, at_tiles[JT - 2], po1)
                bp_mms(JT - 1, at_tiles[JT - 1], po1)
                pending = (q, po1, den)
            q_last, po1_l, den_l = pending
            emit_post(*emit_post_head(q_last, po1_l, den_l))

    _split_waits(nc)
    return nc


def _split_waits(nc):
    """walrus' engine pseudo-instructions accept at most one sync-wait;
    hoist extra waits onto single-wait NoOps on the same engine right
    before the instruction."""
    for f in nc.m.functions:
        for blk in f.blocks:
            new_insts = []
            for inst in blk.instructions:
                si = inst.sync_info
                if si is not None and len(si.on_wait) > 1:
                    waits = list(si.on_wait)
                    for wi, w in enumerate(waits[:-1]):
                        nop = mybir.InstNoOp(
                            name=f"{inst.name}-wsplit{wi}", engine=inst.engine
                        )
                        nop.sync_info = mybir.SyncInfo(on_wait=[w], on_update=[])
                        new_insts.append(nop)
                    inst.sync_info = mybir.SyncInfo(
                        on_wait=waits[-1:], on_update=list(si.on_update)
                    )
                new_insts.append(inst)
            blk.instructions = new_insts


def _tile_rows(a, width):
    """[R(=c*128), M(=s*width)] -> [s, 128, c, width] host relayout so each
    SBUF partition row is one contiguous DRAM run."""
    R, M = a.shape
    c = R // P
    s = M // width
    return np.ascontiguousarray(
        a.reshape(c, P, s, width).transpose(2, 1, 0, 3)
    )


def _prep_core_inputs(values, keys, query, mask, wqk, w2T, bo_rep):
    in_maps = []
    kv_cache = {}
    for c in range(NCORES):
        b, h = divmod(c, 2)
        qs = slice(h * QH, (h + 1) * QH)
        if b not in kv_cache:
            # kT: [d, j] tiled; vN: natural [j, f] rows-to-partitions
            vn = np.ascontiguousarray(
                values[b, 0].astype(BF).reshape(JT, P, KSPAN).transpose(1, 0, 2)
            )
            kv_cache[b] = (
                _tile_rows(np.ascontiguousarray(keys[b, 0].T.astype(BF)), KSPAN),
                vn,
            )
        kTl, vNl = kv_cache[b]
        qTl = _tile_rows(
            np.ascontiguousarray(query[b, 0, qs, :].T.astype(BF)), KSPAN
        )
        m01 = (mask[b, 0, qs, :] != 0).astype(BF)
        # [j, i] -> [q, g, p, t, i]: j = g*512 + t*128 + p, i = q*512 + iw
        mT = np.ascontiguousarray(
            m01.T.reshape(8, 4, P, NQ, IQW).transpose(3, 0, 2, 1, 4)
        )
        in_maps.append(
            {
                "qT": qTl,
                "kT": kTl,
                "vN": vNl,
                "maskT": mT,
                "wqk": wqk,
                "w2T": w2T,
                "bo": bo_rep,
            }
        )
    return in_maps


def kernel(values, keys, query, mask, Wv, Wk, Wq, Wo, bo, _profile=False):
    values = np.asarray(values, dtype=np.float32)
    keys = np.asarray(keys, dtype=np.float32)
    query = np.asarray(query, dtype=np.float32)
    mask = np.asarray(mask)
    Wv = np.asarray(Wv, dtype=np.float32)
    Wk = np.asarray(Wk, dtype=np.float32)
    Wq = np.asarray(Wq, dtype=np.float32)
    Wo = np.asarray(Wo, dtype=np.float32)
    bo = np.asarray(bo, dtype=np.float32)

    scale = np.float32(1.0 / np.sqrt(E))
    # A = Wq'.T @ Wk: scores = q A k.T;  lhsT layout [d(part), d2(free)]
    wqk_m = _tile_rows(
        np.ascontiguousarray(((Wq * scale).T @ Wk).astype(BF)), E
    )[0]
    w2T = _tile_rows(np.ascontiguousarray((Wo @ Wv).T.astype(BF)), E)[0]
    bo_rep = np.ascontiguousarray(
        np.broadcast_to(bo, (P, E)).astype(np.float32)
    )

    in_maps = _prep_core_inputs(values, keys, query, mask, wqk_m, w2T, bo_rep)

    nc = build_bass()
    res = run_bass_kernel_spmd(
        nc, in_maps, core_ids=list(range(NCORES)), trace=_profile
    )

    out = np.empty((N, S, E), dtype=np.float32)
    for c in range(NCORES):
        b, h = divmod(c, 2)
        out[b, h * QH:(h + 1) * QH, :] = res.results[c]["out"].astype(np.float32)

    if _profile:
        return out, res
    return out


if __name__ == "__main__":
    rng = np.random.default_rng(0)
    inputs = {
        "values": rng.standard_normal((N, 1, S, E), dtype=np.float32),
        "keys": rng.standard_normal((N, 1, S, E), dtype=np.float32),
        "query": rng.standard_normal((N, 1, S, E), dtype=np.float32),
        "mask": rng.integers(0, 2, size=(N, 1, S, S)).astype(np.int32),
        "Wv": rng.standard_normal((E, E), dtype=np.float32) / np.sqrt(E),
        "Wk": rng.standard_normal((E, E), dtype=np.float32) / np.sqrt(E),
        "Wq": rng.standard_normal((E, E), dtype=np.float32) / np.sqrt(E),
        "Wo": rng.standard_normal((E, E), dtype=np.float32) / np.sqrt(E),
        "bo": np.zeros((E,), dtype=np.float32),
    }
    out = kernel(**inputs)
    print("out shape:", out.shape, out.dtype)

